# revision 7
# baseline (speedup 1.0000x reference)
"""nn_CrossAttention — Trainium2 Bass kernel (8 NeuronCores, SPMD).

Sharding: core c handles batch b=c//2 and head-group g=c%2 (4 of 8 heads):
data-parallel over batch, tensor-parallel over heads. Each core computes
yT_partial = (softmax(Q_g K_g^T / sqrt(d)) V_g @ Wo_g)^T for its batch.
Host-side unshard sums the two head-group partials per batch, transposes,
and adds the output bias.

On-device layout: everything is computed in the "transposed" domain
(queries on the free dim) so the PE contracts along partitions without any
on-device transposes. Softmax denominators come free from an extra ones
column appended to V (row 64 of the O-matmul PSUM accumulator); the skipped
max-subtraction is safe because scores are O(1) for these inputs.

Score matmuls run in fp8(e4m3) DoubleRow perf mode (0.5 PE cycles/row):
the two DoubleRow contraction slots hold (Q_hi, Q_lo) against a duplicated
fp8 K, i.e. scores = (Q_hi + Q_lo) @ K8 — Q at ~11-bit effective precision,
only K carries fp8 quantization error. Remaining matmuls are float32r.
"""

from contextlib import ExitStack

import numpy as np

import concourse.bass as bass
import concourse.mybir as mybir
import concourse.tile as tile
from concourse import bacc
from concourse.bass_utils import run_bass_kernel_spmd

F32 = mybir.dt.float32
F32R = mybir.dt.float32r
F8 = mybir.dt.float8e4
BF16 = mybir.dt.bfloat16
DR = mybir.MatmulPerfMode.DoubleRow

B, H = 4, 8
N = 4096          # queries per batch
M = 1024          # keys
QD = 1024         # query dim
CD = 768          # context dim
DH = 64           # head dim
HL = 4            # heads per core
IL = HL * DH      # local inner = 256
SCALE = DH ** -0.5

QCH = 512         # query chunk (moving dim)
NCH = N // QCH
NKT = M // 128
NQT = QD // 128
NCT = CD // 128
NYT = QD // 128


def _build(dt_mm=F32R, dt_pt=BF16):
    nc = bacc.Bacc("TRN2", target_bir_lowering=False, debug=False)

    xT = nc.declare_dram_parameter("xT", [QD, N], BF16, isOutput=False)
    ctxT = nc.declare_dram_parameter("ctxT", [CD, M], BF16, isOutput=False)
    wq = nc.declare_dram_parameter("wq", [QD, IL], BF16, isOutput=False)
    wk = nc.declare_dram_parameter("wk", [CD, IL], BF16, isOutput=False)
    wv = nc.declare_dram_parameter("wv", [CD, IL], BF16, isOutput=False)
    wo = nc.declare_dram_parameter("wo", [IL, QD], dt_mm, isOutput=False)
    yT = nc.declare_dram_parameter("yT", [QD, N], BF16, isOutput=True)

    xT_r = xT.rearrange("(kt p) (c q) -> p kt c q", p=128, q=QCH)
    ctx_r = ctxT.rearrange("(ct p) m -> p ct m", p=128)
    wq_r = wq.rearrange("(kt p) i -> p kt i", p=128)
    wk_r = wk.rearrange("(ct p) i -> p ct i", p=128)
    wv_r = wv.rearrange("(ct p) i -> p ct i", p=128)
    wo_r = wo.rearrange("(it p) d -> p it d", p=128)
    yT_r = yT.rearrange("(yt p) (c q) -> p yt c q", p=128, q=QCH)

    with tile.TileContext(nc) as tc, ExitStack() as stack:
        sing = stack.enter_context(tc.tile_pool(name="sing", bufs=1))

        # ---- stage A: load weights, compute K^T (fp8, DR layout) and V_aug --
        # DMA order matters: the HWDGE queue drains in emission order, and the
        # first PE work (K-proj) needs ctx+wk, so those go first.
        kt8_sb = sing.tile([128, 2, 2, M], F8)  # K^T fp8 DR: [dh%128, mi, slot, keys]
        vaug_sb = sing.tile([128, NKT, HL, DH + 1], dt_pt)  # [key%128, kt, head, dh+1]
        wq_sb = sing.tile([128, NQT, IL], BF16)
        wo_sb = sing.tile([128, 2, QD], dt_mm)

        with tc.tile_pool(name="stagea", bufs=1) as stagea, \
             tc.tile_pool(name="psa_a", bufs=2, space="PSUM") as psa_a:
            ctx_sb = stagea.tile([128, NCT, M], BF16)
            nc.sync.dma_start(out=ctx_sb, in_=ctx_r)
            wk_sb = stagea.tile([128, NCT, IL], BF16)
            nc.sync.dma_start(out=wk_sb, in_=wk_r)
            wv_sb = stagea.tile([128, NCT, IL], BF16)
            nc.sync.dma_start(out=wv_sb, in_=wv_r)
            nc.sync.dma_start(out=wq_sb, in_=wq_r)
            nc.sync.dma_start(out=wo_sb, in_=wo_r)
            nc.vector.memset(vaug_sb[:, :, :, DH:DH + 1], 1.0)

            for mi in range(2):
                for nch2 in range(M // QCH):
                    pk = psa_a.tile([128, QCH], F32, tag="ps1")
                    for ct in range(NCT):
                        nc.tensor.matmul(
                            pk, wk_sb[:, ct, mi * 128:(mi + 1) * 128],
                            ctx_sb[:, ct, nch2 * QCH:(nch2 + 1) * QCH],
                            start=(ct == 0), stop=(ct == NCT - 1))
                    for sl in range(2):
                        nc.vector.tensor_copy(
                            kt8_sb[:, mi, sl, nch2 * QCH:(nch2 + 1) * QCH], pk)
            for kt in range(NKT):
                pv = psa_a.tile([128, IL], F32, tag="ps1")
                for ct in range(NCT):
                    nc.tensor.matmul(
                        pv, ctx_sb[:, ct, kt * 128:(kt + 1) * 128],
                        wv_sb[:, ct, :],
                        start=(ct == 0), stop=(ct == NCT - 1))
                nc.vector.tensor_copy(
                    vaug_sb[:, kt, :, 0:DH],
                    pv.rearrange("p (h d) -> p h d", h=HL))

        # ---- stage B pools (opened after stage A space is released) ----
        xpool = stack.enter_context(tc.tile_pool(name="xpool", bufs=2))
        qtp = stack.enter_context(tc.tile_pool(name="qtp", bufs=2))
        ptp = stack.enter_context(tc.tile_pool(name="ptp", bufs=2))
        o2p = stack.enter_context(tc.tile_pool(name="o2p", bufs=3))
        ypool = stack.enter_context(tc.tile_pool(name="ypool", bufs=2))
        smallp = stack.enter_context(tc.tile_pool(name="smallp", bufs=2))
        pss = stack.enter_context(tc.tile_pool(name="pss", bufs=2, space="PSUM"))
        pso = stack.enter_context(tc.tile_pool(name="pso", bufs=1, space="PSUM"))
        psqy = stack.enter_context(tc.tile_pool(name="psqy", bufs=2, space="PSUM"))

        for c in range(NCH):
            xc = xpool.tile([128, NQT, QCH], BF16)
            nc.sync.dma_start(out=xc, in_=xT_r[:, :, c, :])

            # Q in fp8 hi/lo DoubleRow layout: [dh%128, mi, slot, q]
            qt8 = qtp.tile([128, 2, 2, QCH], F8)
            for mi in range(2):
                pq = psqy.tile([128, QCH], F32, tag="qy")
                for kt in range(NQT):
                    nc.tensor.matmul(
                        pq, wq_sb[:, kt, mi * 128:(mi + 1) * 128],
                        xc[:, kt, :],
                        start=(kt == 0), stop=(kt == NQT - 1))
                nc.vector.tensor_copy(qt8[:, mi, 0, :], pq)
                nc.vector.tensor_sub(qt8[:, mi, 1, :], pq, qt8[:, mi, 0, :])

            o2t = []
            for mi in range(2):   # head pairs: heads (2mi, 2mi+1)
                # probs for both heads of the pair: [p, kt, head, q]
                pt = ptp.tile([128, NKT, 2, QCH], dt_pt, tag=f"pt{mi}")
                for kt in range(NKT):
                    ps2 = pss.tile([128, 2 * QCH], F32, tag="ps")
                    nc.tensor.matmul(
                        ps2[:, 0:QCH],
                        kt8_sb[0:64, mi, :, kt * 128:(kt + 1) * 128],
                        qt8[0:64, mi, :, :], start=True, stop=True,
                        perf_mode=DR)
                    nc.tensor.matmul(
                        ps2[:, QCH:2 * QCH],
                        kt8_sb[64:128, mi, :, kt * 128:(kt + 1) * 128],
                        qt8[64:128, mi, :, :], start=True, stop=True,
                        perf_mode=DR)
                    nc.scalar.activation(pt[:, kt, :, :], ps2,
                                         mybir.ActivationFunctionType.Exp,
                                         scale=SCALE)
                poa = pso.tile([DH + 1, QCH], F32, tag="poa")
                pob = pso.tile([DH + 1, QCH], F32, tag="pob")
                for kt in range(NKT):
                    nc.tensor.matmul(poa, vaug_sb[:, kt, 2 * mi, :],
                                     pt[:, kt, 0, :],
                                     start=(kt == 0), stop=(kt == NKT - 1))
                    nc.tensor.matmul(pob, vaug_sb[:, kt, 2 * mi + 1, :],
                                     pt[:, kt, 1, :],
                                     start=(kt == 0), stop=(kt == NKT - 1))
                ra = smallp.tile([1, QCH], F32, tag="ra")
                rb = smallp.tile([1, QCH], F32, tag="rb")
                nc.vector.reciprocal(ra, poa[DH:DH + 1, :])
                nc.vector.reciprocal(rb, pob[DH:DH + 1, :])
                bca = smallp.tile([64, QCH], F32, tag="bca")
                bcb = smallp.tile([64, QCH], F32, tag="bcb")
                nc.gpsimd.partition_broadcast(bca, ra)
                nc.gpsimd.partition_broadcast(bcb, rb)
                ot = o2p.tile([128, QCH], dt_mm)
                nc.vector.tensor_mul(ot[0:64, :], poa[0:DH, :], bca)
                nc.vector.tensor_mul(ot[64:128, :], pob[0:DH, :], bcb)
                o2t.append(ot)

            yc = ypool.tile([128, NYT, QCH], BF16)
            for yt in range(NYT):
                py = psqy.tile([128, QCH], F32, tag="qy")
                nc.tensor.matmul(py, wo_sb[:, 0, yt * 128:(yt + 1) * 128],
                                 o2t[0], start=True, stop=False)
                nc.tensor.matmul(py, wo_sb[:, 1, yt * 128:(yt + 1) * 128],
                                 o2t[1], start=False, stop=True)
                nc.vector.tensor_copy(yc[:, yt, :], py)
            nc.sync.dma_start(out=yT_r[:, :, c, :], in_=yc)

    nc.compile()
    return nc


_NC_CACHE = {}


def _get_nc():
    if "nc" not in _NC_CACHE:
        _NC_CACHE["nc"] = _build()
    return _NC_CACHE["nc"]


def kernel(x, context, Wq, Wk, Wv, Wo, bo):
    import ml_dtypes
    bf = ml_dtypes.bfloat16
    x = np.asarray(x, np.float32)
    context = np.asarray(context, np.float32)
    Wq = np.asarray(Wq, np.float32)
    Wk = np.asarray(Wk, np.float32)
    Wv = np.asarray(Wv, np.float32)
    Wo = np.asarray(Wo, np.float32)
    bo = np.asarray(bo, np.float32)

    nc = _get_nc()
    in_maps = []
    for c in range(8):
        b, g = c // 2, c % 2
        sl = slice(g * IL, (g + 1) * IL)
        in_maps.append({
            "xT": np.ascontiguousarray(x[b].T).astype(bf),
            "ctxT": np.ascontiguousarray(context[b].T).astype(bf),
            "wq": np.ascontiguousarray(Wq[:, sl]).astype(bf),
            "wk": np.ascontiguousarray(Wk[:, sl]).astype(bf),
            "wv": np.ascontiguousarray(Wv[:, sl]).astype(bf),
            "wo": np.ascontiguousarray(Wo[sl, :]),
        })

    res = None
    for attempt in range(3):
        try:
            res = run_bass_kernel_spmd(nc, in_maps, core_ids=list(range(8)))
            if any(np.isnan(r["yT"].astype(np.float32)).any()
                   for r in res.results):
                raise RuntimeError("NaN in device output")
            break
        except Exception:
            # the axon-tunneled device occasionally reports
            # NRT_EXEC_UNIT_UNRECOVERABLE; the failure sticks to the PJRT
            # client, so tear down the backend to get a fresh worker
            if attempt == 2:
                raise
            import time
            import jax
            time.sleep(10)
            try:
                jax.clear_caches()
                jax.extend.backend.clear_backends()
            except Exception:
                pass
    ys = []
    for b in range(B):
        yt = (res.results[2 * b]["yT"].astype(np.float32)
              + res.results[2 * b + 1]["yT"].astype(np.float32))
        ys.append(yt.T + bo[None, :])
    return np.stack(ys, 0).astype(np.float32)


# revision 12
# speedup vs baseline: 1.0879x; 1.0879x over previous
"""nn_CrossAttention — Trainium2 Bass kernel (8 NeuronCores, SPMD).

Sharding: core c handles batch b=c//2 and head-group g=c%2 (4 of 8 heads):
data-parallel over batch, tensor-parallel over heads. Each core computes
yT_partial = (softmax(Q_g K_g^T / sqrt(d)) V_g @ Wo_g)^T for its batch.
Host-side unshard sums the two head-group partials per batch, transposes,
and adds the output bias.

On-device layout: everything is computed in the "transposed" domain
(queries on the free dim) so the PE contracts along partitions without any
on-device transposes. Softmax denominators come free from an extra ones
column appended to V (row 64 of the O-matmul PSUM accumulator); the skipped
max-subtraction is safe because scores are O(1) for these inputs.

Score matmuls run in fp8(e4m3) DoubleRow perf mode (0.5 PE cycles/row):
the two DoubleRow contraction slots hold (Q_hi, Q_lo) against a duplicated
fp8 K, i.e. scores = (Q_hi + Q_lo) @ K8 — Q at ~11-bit effective precision,
only K carries fp8 quantization error. Remaining matmuls are float32r.
"""

from contextlib import ExitStack

import numpy as np

import concourse.bass as bass
import concourse.mybir as mybir
import concourse.tile as tile
from concourse import bacc
from concourse.bass_utils import run_bass_kernel_spmd

F32 = mybir.dt.float32
F32R = mybir.dt.float32r
F8 = mybir.dt.float8e4
BF16 = mybir.dt.bfloat16
DR = mybir.MatmulPerfMode.DoubleRow

B, H = 4, 8
N = 4096          # queries per batch
M = 1024          # keys
QD = 1024         # query dim
CD = 768          # context dim
DH = 64           # head dim
HL = 4            # heads per core
IL = HL * DH      # local inner = 256
SCALE = DH ** -0.5

QCH = 512         # query chunk (moving dim)
NCH = N // QCH
NKT = M // 128
NQT = QD // 128
NCT = CD // 128
NYT = QD // 128


def _build(dt_mm=F32R, dt_pt=BF16):
    nc = bacc.Bacc("TRN2", target_bir_lowering=False, debug=False)

    xT = nc.declare_dram_parameter("xT", [QD, N], BF16, isOutput=False)
    ctxT = nc.declare_dram_parameter("ctxT", [CD, M], BF16, isOutput=False)
    wq = nc.declare_dram_parameter("wq", [QD, IL], BF16, isOutput=False)
    wk = nc.declare_dram_parameter("wk", [CD, IL], BF16, isOutput=False)
    wv = nc.declare_dram_parameter("wv", [CD, IL], BF16, isOutput=False)
    wo = nc.declare_dram_parameter("wo", [IL, QD], BF16, isOutput=False)
    yT = nc.declare_dram_parameter("yT", [QD, N], BF16, isOutput=True)

    xT_r = xT.rearrange("(kt p) (c q) -> p kt c q", p=128, q=QCH)
    ctx_r = ctxT.rearrange("(ct p) m -> p ct m", p=128)
    wq_r = wq.rearrange("(kt p) i -> p kt i", p=128)
    wk_r = wk.rearrange("(ct p) i -> p ct i", p=128)
    wv_r = wv.rearrange("(ct p) i -> p ct i", p=128)
    wo_r = wo.rearrange("(it p) d -> p it d", p=128)
    yT_r = yT.rearrange("(yt p) (c q) -> p yt c q", p=128, q=QCH)

    with tile.TileContext(nc) as tc, ExitStack() as stack:
        sing = stack.enter_context(tc.tile_pool(name="sing", bufs=1))

        # ---- stage A: load weights, compute K^T (fp8, DR layout) and V_aug --
        # DMA order matters: the HWDGE queue drains in emission order, and the
        # first PE work (K-proj) needs ctx+wk, so those go first.
        kt8_sb = sing.tile([128, 2, 2, M], F8)  # K^T fp8 DR: [dh%128, mi, slot, keys]
        vaug_sb = sing.tile([128, NKT, HL, DH + 1], dt_pt)  # [key%128, kt, head, dh+1]
        wq_sb = sing.tile([128, NQT, IL], BF16)
        wo_sb = sing.tile([128, 2, QD], BF16)

        with tc.tile_pool(name="stagea", bufs=1) as stagea, \
             tc.tile_pool(name="psa_a", bufs=2, space="PSUM") as psa_a:
            ctx_sb = stagea.tile([128, NCT, M], BF16)
            nc.sync.dma_start(out=ctx_sb, in_=ctx_r)
            wk_sb = stagea.tile([128, NCT, IL], BF16)
            nc.sync.dma_start(out=wk_sb, in_=wk_r)
            wv_sb = stagea.tile([128, NCT, IL], BF16)
            nc.sync.dma_start(out=wv_sb, in_=wv_r)
            nc.sync.dma_start(out=wq_sb, in_=wq_r)
            nc.sync.dma_start(out=wo_sb, in_=wo_r)
            nc.vector.memset(vaug_sb[:, :, :, DH:DH + 1], 1.0)

            for mi in range(2):
                for nch2 in range(M // QCH):
                    pk = psa_a.tile([128, QCH], F32, tag="ps1")
                    for ct in range(NCT):
                        nc.tensor.matmul(
                            pk, wk_sb[:, ct, mi * 128:(mi + 1) * 128],
                            ctx_sb[:, ct, nch2 * QCH:(nch2 + 1) * QCH],
                            start=(ct == 0), stop=(ct == NCT - 1))
                    for sl in range(2):
                        nc.vector.tensor_copy(
                            kt8_sb[:, mi, sl, nch2 * QCH:(nch2 + 1) * QCH], pk)
            for kt in range(NKT):
                pv = psa_a.tile([128, IL], F32, tag="ps1")
                for ct in range(NCT):
                    nc.tensor.matmul(
                        pv, ctx_sb[:, ct, kt * 128:(kt + 1) * 128],
                        wv_sb[:, ct, :],
                        start=(ct == 0), stop=(ct == NCT - 1))
                nc.vector.tensor_copy(
                    vaug_sb[:, kt, :, 0:DH],
                    pv.rearrange("p (h d) -> p h d", h=HL))

        # ---- stage B pools (opened after stage A space is released) ----
        xpool = stack.enter_context(tc.tile_pool(name="xpool", bufs=2))
        qtp = stack.enter_context(tc.tile_pool(name="qtp", bufs=2))
        ptp = stack.enter_context(tc.tile_pool(name="ptp", bufs=2))
        obp = stack.enter_context(tc.tile_pool(name="obp", bufs=3))
        otp = stack.enter_context(tc.tile_pool(name="otp", bufs=2))
        ypool = stack.enter_context(tc.tile_pool(name="ypool", bufs=2))
        smallp = stack.enter_context(tc.tile_pool(name="smallp", bufs=3))
        pss = stack.enter_context(tc.tile_pool(name="pss", bufs=2, space="PSUM"))
        pso = stack.enter_context(tc.tile_pool(name="pso", bufs=2, space="PSUM"))
        psq = stack.enter_context(tc.tile_pool(name="psq", bufs=1, space="PSUM"))
        psy = stack.enter_context(tc.tile_pool(name="psy", bufs=1, space="PSUM"))

        for c in range(NCH):
            xc = xpool.tile([128, NQT, QCH], BF16)
            nc.sync.dma_start(out=xc, in_=xT_r[:, :, c, :])

            # Q in fp8 hi/lo DoubleRow layout: [dh%128, mi, slot, q]
            qt8 = qtp.tile([128, 2, 2, QCH], F8)
            for mi in range(2):
                pq = psq.tile([128, QCH], F32, tag="ps1")
                for kt in range(NQT):
                    nc.tensor.matmul(
                        pq, wq_sb[:, kt, mi * 128:(mi + 1) * 128],
                        xc[:, kt, :],
                        start=(kt == 0), stop=(kt == NQT - 1))
                nc.vector.tensor_copy(qt8[:, mi, 0, :], pq)
                nc.vector.tensor_sub(qt8[:, mi, 1, :], pq, qt8[:, mi, 0, :])

            pts = []
            for mi in range(2):   # head pairs: heads (2mi, 2mi+1)
                # probs for both heads of the pair: [p, kt, head, q]
                pt = ptp.tile([128, NKT, 2, QCH], dt_pt, tag=f"pt{mi}")
                for kt in range(NKT):
                    ps2 = pss.tile([128, 2 * QCH], F32, tag="ps")
                    nc.tensor.matmul(
                        ps2[:, 0:QCH],
                        kt8_sb[0:64, mi, :, kt * 128:(kt + 1) * 128],
                        qt8[0:64, mi, :, :], start=True, stop=True,
                        perf_mode=DR)
                    nc.tensor.matmul(
                        ps2[:, QCH:2 * QCH],
                        kt8_sb[64:128, mi, :, kt * 128:(kt + 1) * 128],
                        qt8[64:128, mi, :, :], start=True, stop=True,
                        perf_mode=DR)
                    nc.scalar.activation(pt[:, kt, :, :], ps2,
                                         mybir.ActivationFunctionType.Exp,
                                         scale=SCALE)
                pts.append(pt)

            # Flipped AV: probs are the stationary operand ([keys, 128
            # queries] slices), V_aug the moving one (65 rows), so queries
            # land on PSUM partitions and the softmax denominator is a
            # per-partition scalar. The normalized [q, d] block is turned
            # back into [d, q] by a DMA-engine transpose.
            oT = otp.tile([128, 2, QCH], BF16)   # [dh-pair, mi, q]
            for mi in range(2):
                pt = pts[mi]
                for qs in range(QCH // 128):
                    po = pso.tile([128, 2 * (DH + 1)], F32, tag="po")
                    pr = po.rearrange("p (h x) -> p h x", h=2)
                    for h01 in range(2):
                        for kt in range(NKT):
                            nc.tensor.matmul(
                                pr[:, h01, :],
                                pt[:, kt, h01, qs * 128:(qs + 1) * 128],
                                vaug_sb[:, kt, 2 * mi + h01, :],
                                start=(kt == 0), stop=(kt == NKT - 1))
                    rp = smallp.tile([128, 2, 1], F32, tag="rp")
                    nc.vector.reciprocal(rp, pr[:, :, DH:DH + 1])
                    ob = obp.tile([128, 2, DH], BF16, tag="ob")
                    nc.vector.tensor_mul(ob, pr[:, :, 0:DH],
                                         rp.broadcast_to([128, 2, DH]))
                    nc.sync.dma_start(
                        out=oT[:, mi, qs * 128:(qs + 1) * 128],
                        in_=ob.rearrange("p h d -> p (h d)"),
                        transpose=True)

            yc = ypool.tile([128, NYT, QCH], BF16)
            for yt in range(NYT):
                py = psy.tile([128, QCH], F32)
                nc.tensor.matmul(py, wo_sb[:, 0, yt * 128:(yt + 1) * 128],
                                 oT[:, 0, :], start=True, stop=False)
                nc.tensor.matmul(py, wo_sb[:, 1, yt * 128:(yt + 1) * 128],
                                 oT[:, 1, :], start=False, stop=True)
                nc.vector.tensor_copy(yc[:, yt, :], py)
            nc.sync.dma_start(out=yT_r[:, :, c, :], in_=yc)

    nc.compile()
    return nc


_NC_CACHE = {}


def _get_nc():
    if "nc" not in _NC_CACHE:
        _NC_CACHE["nc"] = _build()
    return _NC_CACHE["nc"]


def kernel(x, context, Wq, Wk, Wv, Wo, bo):
    import ml_dtypes
    bf = ml_dtypes.bfloat16
    x = np.asarray(x, np.float32)
    context = np.asarray(context, np.float32)
    Wq = np.asarray(Wq, np.float32)
    Wk = np.asarray(Wk, np.float32)
    Wv = np.asarray(Wv, np.float32)
    Wo = np.asarray(Wo, np.float32)
    bo = np.asarray(bo, np.float32)

    nc = _get_nc()
    in_maps = []
    for c in range(8):
        b, g = c // 2, c % 2
        sl = slice(g * IL, (g + 1) * IL)
        in_maps.append({
            "xT": np.ascontiguousarray(x[b].T).astype(bf),
            "ctxT": np.ascontiguousarray(context[b].T).astype(bf),
            "wq": np.ascontiguousarray(Wq[:, sl]).astype(bf),
            "wk": np.ascontiguousarray(Wk[:, sl]).astype(bf),
            "wv": np.ascontiguousarray(Wv[:, sl]).astype(bf),
            "wo": np.ascontiguousarray(Wo[sl, :]).astype(bf),
        })

    res = None
    for attempt in range(3):
        try:
            res = run_bass_kernel_spmd(nc, in_maps, core_ids=list(range(8)))
            if any(np.isnan(r["yT"].astype(np.float32)).any()
                   for r in res.results):
                raise RuntimeError("NaN in device output")
            break
        except Exception:
            # the axon-tunneled device occasionally reports
            # NRT_EXEC_UNIT_UNRECOVERABLE; the failure sticks to the PJRT
            # client, so tear down the backend to get a fresh worker
            if attempt == 2:
                raise
            import time
            import jax
            time.sleep(10)
            try:
                jax.clear_caches()
                jax.extend.backend.clear_backends()
            except Exception:
                pass
    ys = []
    for b in range(B):
        yt = (res.results[2 * b]["yT"].astype(np.float32)
              + res.results[2 * b + 1]["yT"].astype(np.float32))
        ys.append(yt.T + bo[None, :])
    return np.stack(ys, 0).astype(np.float32)


# revision 15
# speedup vs baseline: 1.5431x; 1.4184x over previous
"""nn_CrossAttention — Trainium2 Bass kernel (8 NeuronCores, SPMD).

Sharding: core c handles batch b=c//2 and head-group g=c%2 (4 of 8 heads):
data-parallel over batch, tensor-parallel over heads. Each core computes
yT_partial = (softmax(Q_g K_g^T / sqrt(d)) V_g @ Wo_g)^T for its batch.
Host-side unshard sums the two head-group partials per batch, transposes,
and adds the output bias.

On-device layout: everything is computed in the "transposed" domain
(queries on the free dim) so the PE contracts along partitions without any
on-device transposes. Softmax denominators come free from an extra ones
column appended to V (row 64 of the O-matmul PSUM accumulator); the skipped
max-subtraction is safe because scores are O(1) for these inputs.

Score matmuls run in fp8(e4m3) DoubleRow perf mode (0.5 PE cycles/row):
the two DoubleRow contraction slots hold (Q_hi, Q_lo) against a duplicated
fp8 K, i.e. scores = (Q_hi + Q_lo) @ K8 — Q at ~11-bit effective precision,
only K carries fp8 quantization error. Everything else runs in bf16 except
the fp32 PSUM accumulations.

The chunk loop is software-pipelined around the Activation engine (exp is
the critical resource): chunk c's score matmuls + exps are emitted first,
then Wo of chunk c-1 and the Q projection of chunk c+1 fill the PE while
the exps drain, then the AV matmuls (which consume the exps) close chunk c.
Chunk 0's Q/scores are interleaved into stage A so the exp pipeline starts
as soon as K exists, with the V projection running under the first exps.
"""

from contextlib import ExitStack

import numpy as np

import concourse.bass as bass
import concourse.mybir as mybir
import concourse.tile as tile
from concourse import bacc
from concourse.bass_utils import run_bass_kernel_spmd

F32 = mybir.dt.float32
F8 = mybir.dt.float8e4
BF16 = mybir.dt.bfloat16
DR = mybir.MatmulPerfMode.DoubleRow

B, H = 4, 8
N = 4096          # queries per batch
M = 1024          # keys
QD = 1024         # query dim
CD = 768          # context dim
DH = 64           # head dim
HL = 4            # heads per core
IL = HL * DH      # local inner = 256
SCALE = DH ** -0.5

QCH = 512         # query chunk (moving dim)
NCH = N // QCH
NKT = M // 128
NQT = QD // 128
NCT = CD // 128
NYT = QD // 128


def _build():
    nc = bacc.Bacc("TRN2", target_bir_lowering=False, debug=False)

    xT = nc.declare_dram_parameter("xT", [QD, N], BF16, isOutput=False)
    ctxT = nc.declare_dram_parameter("ctxT", [CD, M], BF16, isOutput=False)
    wq = nc.declare_dram_parameter("wq", [QD, IL], BF16, isOutput=False)
    wk = nc.declare_dram_parameter("wk", [CD, IL], BF16, isOutput=False)
    wv = nc.declare_dram_parameter("wv", [CD, IL], BF16, isOutput=False)
    wo = nc.declare_dram_parameter("wo", [IL, QD], BF16, isOutput=False)
    yT = nc.declare_dram_parameter("yT", [QD, N], BF16, isOutput=True)

    xT_r = xT.rearrange("(kt p) (c q) -> p kt c q", p=128, q=QCH)
    ctx_r = ctxT.rearrange("(ct p) m -> p ct m", p=128)
    wq_r = wq.rearrange("(kt p) i -> p kt i", p=128)
    wk_r = wk.rearrange("(ct p) i -> p ct i", p=128)
    wv_r = wv.rearrange("(ct p) i -> p ct i", p=128)
    wo_r = wo.rearrange("(it p) d -> p it d", p=128)
    yT_r = yT.rearrange("(yt p) (c q) -> p yt c q", p=128, q=QCH)

    with tile.TileContext(nc) as tc, ExitStack() as stack:
        sing = stack.enter_context(tc.tile_pool(name="sing", bufs=1))

        kt8_sb = sing.tile([128, 2, 2, M], F8)  # K^T fp8 DR: [dh%128, mi, slot, keys]
        vaug_sb = sing.tile([128, NKT, HL, DH + 1], BF16)  # [key%128, kt, head, dh+1]
        wq_sb = sing.tile([128, NQT, IL], BF16)
        wo_sb = sing.tile([128, 2, QD], BF16)

        # stage B pools — declared up front so chunk 0 can interleave with
        # stage A. The stage-A projections borrow psq/psy as accumulators.
        xpool = stack.enter_context(tc.tile_pool(name="xpool", bufs=2))
        qtp = stack.enter_context(tc.tile_pool(name="qtp", bufs=2))
        ptp = stack.enter_context(tc.tile_pool(name="ptp", bufs=2))
        o2p = stack.enter_context(tc.tile_pool(name="o2p", bufs=2))
        ypool = stack.enter_context(tc.tile_pool(name="ypool", bufs=2))
        smallp = stack.enter_context(tc.tile_pool(name="smallp", bufs=2))
        pss = stack.enter_context(tc.tile_pool(name="pss", bufs=2, space="PSUM"))
        pso = stack.enter_context(tc.tile_pool(name="pso", bufs=1, space="PSUM"))
        psq = stack.enter_context(tc.tile_pool(name="psq", bufs=1, space="PSUM"))
        psy = stack.enter_context(tc.tile_pool(name="psy", bufs=1, space="PSUM"))

        def load_x(c):
            xc = xpool.tile([128, NQT, QCH], BF16, tag="xc")
            nc.sync.dma_start(out=xc, in_=xT_r[:, :, c, :])
            return xc

        def make_q(c, xc):
            # Q in fp8 hi/lo DoubleRow layout: [dh%128, mi, slot, q]
            qt8 = qtp.tile([128, 2, 2, QCH], F8, tag="qt8")
            for mi in range(2):
                pq = psq.tile([128, QCH], F32, tag="ps1")
                for kt in range(NQT):
                    nc.tensor.matmul(
                        pq, wq_sb[:, kt, mi * 128:(mi + 1) * 128],
                        xc[:, kt, :],
                        start=(kt == 0), stop=(kt == NQT - 1))
                nc.vector.tensor_copy(qt8[:, mi, 0, :], pq)
                nc.vector.tensor_sub(qt8[:, mi, 1, :], pq, qt8[:, mi, 0, :])
            return qt8

        def scores(c, qt8, mi):
            # probs for both heads of the pair: [p, kt, head, q]
            pt = ptp.tile([128, NKT, 2, QCH], BF16, tag=f"pt{mi}")
            for kt in range(NKT):
                ps2 = pss.tile([128, 2 * QCH], F32, tag="ps")
                nc.tensor.matmul(
                    ps2[:, 0:QCH],
                    kt8_sb[0:64, mi, :, kt * 128:(kt + 1) * 128],
                    qt8[0:64, mi, :, :], start=True, stop=True,
                    perf_mode=DR)
                nc.tensor.matmul(
                    ps2[:, QCH:2 * QCH],
                    kt8_sb[64:128, mi, :, kt * 128:(kt + 1) * 128],
                    qt8[64:128, mi, :, :], start=True, stop=True,
                    perf_mode=DR)
                nc.scalar.activation(pt[:, kt, :, :], ps2,
                                     mybir.ActivationFunctionType.Exp,
                                     scale=SCALE)
            return pt

        def av(c, pt, mi):
            poa = pso.tile([DH + 1, QCH], F32, tag="poa")
            pob = pso.tile([DH + 1, QCH], F32, tag="pob")
            for kt in range(NKT):
                nc.tensor.matmul(poa, vaug_sb[:, kt, 2 * mi, :],
                                 pt[:, kt, 0, :],
                                 start=(kt == 0), stop=(kt == NKT - 1))
                nc.tensor.matmul(pob, vaug_sb[:, kt, 2 * mi + 1, :],
                                 pt[:, kt, 1, :],
                                 start=(kt == 0), stop=(kt == NKT - 1))
            ra = smallp.tile([1, QCH], F32, tag="ra")
            rb = smallp.tile([1, QCH], F32, tag="rb")
            nc.vector.reciprocal(ra, poa[DH:DH + 1, :])
            nc.vector.reciprocal(rb, pob[DH:DH + 1, :])
            bca = smallp.tile([64, QCH], F32, tag="bca")
            bcb = smallp.tile([64, QCH], F32, tag="bcb")
            nc.gpsimd.partition_broadcast(bca, ra)
            nc.gpsimd.partition_broadcast(bcb, rb)
            ot = o2p.tile([128, QCH], BF16, tag=f"ot{mi}")
            nc.vector.tensor_mul(ot[0:64, :], poa[0:DH, :], bca)
            nc.vector.tensor_mul(ot[64:128, :], pob[0:DH, :], bcb)
            return ot

        def wo_out(c, o2t):
            yc = ypool.tile([128, NYT, QCH], BF16, tag="yc")
            for yt in range(NYT):
                py = psy.tile([128, QCH], F32, tag="ps1")
                nc.tensor.matmul(py, wo_sb[:, 0, yt * 128:(yt + 1) * 128],
                                 o2t[0], start=True, stop=False)
                nc.tensor.matmul(py, wo_sb[:, 1, yt * 128:(yt + 1) * 128],
                                 o2t[1], start=False, stop=True)
                nc.vector.tensor_copy(yc[:, yt, :], py)
                if yt % 2 == 1:   # drain y in quarters to shrink the tail
                    nc.sync.dma_start(out=yT_r[:, yt - 1:yt + 1, c, :],
                                      in_=yc[:, yt - 1:yt + 1, :])

        # ---- stage A interleaved with chunk 0 ----
        # DMA order: first ctx half + wk unblock K-proj; x0/wq unblock the
        # chunk-0 Q projection; the rest follow.
        with tc.tile_pool(name="stagea", bufs=1) as stagea:
            ctx_sb = stagea.tile([128, NCT, M], BF16)
            nc.sync.dma_start(out=ctx_sb[:, :, 0:QCH], in_=ctx_r[:, :, 0:QCH])
            wk_sb = stagea.tile([128, NCT, IL], BF16)
            nc.sync.dma_start(out=wk_sb, in_=wk_r)
            nc.sync.dma_start(out=ctx_sb[:, :, QCH:M], in_=ctx_r[:, :, QCH:M])
            nc.sync.dma_start(out=wq_sb, in_=wq_r)
            xc = load_x(0)
            wv_sb = stagea.tile([128, NCT, IL], BF16)
            nc.sync.dma_start(out=wv_sb, in_=wv_r)
            nc.sync.dma_start(out=wo_sb, in_=wo_r)
            xn = load_x(1)
            nc.vector.memset(vaug_sb[:, :, :, DH:DH + 1], 1.0)

            # K projection (psq/psy banks alternate as accumulators)
            for mi in range(2):
                for nch2 in range(M // QCH):
                    pool_k = psq if (mi * 2 + nch2) % 2 == 0 else psy
                    pk = pool_k.tile([128, QCH], F32, tag="ps1")
                    for ct in range(NCT):
                        nc.tensor.matmul(
                            pk, wk_sb[:, ct, mi * 128:(mi + 1) * 128],
                            ctx_sb[:, ct, nch2 * QCH:(nch2 + 1) * QCH],
                            start=(ct == 0), stop=(ct == NCT - 1))
                    for sl in range(2):
                        nc.vector.tensor_copy(
                            kt8_sb[:, mi, sl, nch2 * QCH:(nch2 + 1) * QCH], pk)

            # chunk 0 Q + first score pair start the exp pipeline early
            qt8 = make_q(0, xc)
            pt00 = scores(0, qt8, 0)

            # V projection runs on the PE while the ACT engine exps pair 0
            for kt in range(NKT):
                pool_v = psq if kt % 2 == 0 else psy
                pv = pool_v.tile([128, IL], F32, tag="ps1")
                for ct in range(NCT):
                    nc.tensor.matmul(
                        pv, ctx_sb[:, ct, kt * 128:(kt + 1) * 128],
                        wv_sb[:, ct, :],
                        start=(ct == 0), stop=(ct == NCT - 1))
                nc.vector.tensor_copy(
                    vaug_sb[:, kt, :, 0:DH],
                    pv.rearrange("p (h d) -> p h d", h=HL))

        # ---- software-pipelined chunk loop ----
        pts = [pt00, scores(0, qt8, 1)]
        prev_o2t = None
        for c in range(NCH):
            if c > 0:
                pts = [scores(c, qt8, 0), scores(c, qt8, 1)]
            if prev_o2t is not None:
                wo_out(c - 1, prev_o2t)
            if c + 1 < NCH:
                qt8 = make_q(c + 1, xn)
            if c + 2 < NCH:
                xn = load_x(c + 2)
            prev_o2t = [av(c, pts[0], 0), av(c, pts[1], 1)]
        wo_out(NCH - 1, prev_o2t)

    nc.compile()
    return nc


_NC_CACHE = {}


def _get_nc():
    if "nc" not in _NC_CACHE:
        _NC_CACHE["nc"] = _build()
    return _NC_CACHE["nc"]


def kernel(x, context, Wq, Wk, Wv, Wo, bo):
    import ml_dtypes
    bf = ml_dtypes.bfloat16
    x = np.asarray(x, np.float32)
    context = np.asarray(context, np.float32)
    Wq = np.asarray(Wq, np.float32)
    Wk = np.asarray(Wk, np.float32)
    Wv = np.asarray(Wv, np.float32)
    Wo = np.asarray(Wo, np.float32)
    bo = np.asarray(bo, np.float32)

    nc = _get_nc()
    in_maps = []
    for c in range(8):
        b, g = c // 2, c % 2
        sl = slice(g * IL, (g + 1) * IL)
        in_maps.append({
            "xT": np.ascontiguousarray(x[b].T).astype(bf),
            "ctxT": np.ascontiguousarray(context[b].T).astype(bf),
            "wq": np.ascontiguousarray(Wq[:, sl]).astype(bf),
            "wk": np.ascontiguousarray(Wk[:, sl]).astype(bf),
            "wv": np.ascontiguousarray(Wv[:, sl]).astype(bf),
            "wo": np.ascontiguousarray(Wo[sl, :]).astype(bf),
        })

    res = None
    for attempt in range(3):
        try:
            res = run_bass_kernel_spmd(nc, in_maps, core_ids=list(range(8)))
            if any(np.isnan(r["yT"].astype(np.float32)).any()
                   for r in res.results):
                raise RuntimeError("NaN in device output")
            break
        except Exception:
            # the axon-tunneled device occasionally reports
            # NRT_EXEC_UNIT_UNRECOVERABLE; the failure sticks to the PJRT
            # client, so tear down the backend to get a fresh worker
            if attempt == 2:
                raise
            import time
            import jax
            time.sleep(10)
            try:
                jax.clear_caches()
                jax.extend.backend.clear_backends()
            except Exception:
                pass
    ys = []
    for b in range(B):
        yt = (res.results[2 * b]["yT"].astype(np.float32)
              + res.results[2 * b + 1]["yT"].astype(np.float32))
        ys.append(yt.T + bo[None, :])
    return np.stack(ys, 0).astype(np.float32)
